# revision 1
# baseline (speedup 1.0000x reference)
"""Entmax-1.5 over rows of a (2048, 32000) fp32 tensor on 8 Trainium2 NeuronCores.

Algorithm (per row): find tau s.t. sum(relu((x - max)/2 - tau)^2) == 1, then
Y = relu((x-max)/2 - tau)^2.  Instead of the reference's full sort:
  1. chunk-max (chunks of 16) -> M[2000]; row max = top-1
  2. top-24 chunk maxima via DVE max8 + match_replace
  3. warm-start Newton solve of sum(relu2)=1 on those 24 values only
     (a lower bound of the true objective -> tau0 <= tau*)
  4. one full-width Newton iteration (DVE scalar_tensor_tensor relu with
     sum accumulator; ScalarE Square(0.5*x) accumulators for f), then a
     guarded secant iteration (relu update + Square pass only)
  5. output pass: Y = Square(0.5*r1 - 0.5*dX2) -- the final relu is
     folded into the Square bias; for clipped elements the reference is 0
     and the introduced error is <= (0.5*dX2)^2 ~ 2e-6.

All relu updates are in-place and shift-invariant:
relu(relu(x-a)-b) = relu(x-a-b), so the original X is consumed once.
Each row block lives as 8 column tiles in a 10-deep pool: two of the next
block's tiles prefetch during the current block's iterations, the rest
chain tile-by-tile behind the output DMAs.

Sharding: pure data parallel over rows; core i handles rows [256*i, 256*(i+1)).
"""

import numpy as np

import concourse.bass as bass
import concourse.bacc as bacc
import concourse.mybir as mybir
from concourse.tile import TileContext
from concourse.bass_utils import run_bass_kernel_spmd

f32 = mybir.dt.float32
Alu = mybir.AluOpType
Act = mybir.ActivationFunctionType
AxX = mybir.AxisListType.X

ROWS_TOTAL = 2048
V = 32000
N_CORES = 8
ROWS_PER_CORE = ROWS_TOTAL // N_CORES  # 256
P = 128
N_BLOCKS = ROWS_PER_CORE // P          # 2
CHUNK = 32
NCHUNKS = V // CHUNK                   # 1000
TOPK_ROUNDS = 3                        # top-24 chunk maxima
TOPK = 8 * TOPK_ROUNDS
WARM_ITERS = 8
COL_TILE = 4000                        # column tile (DMA + pass granularity)
NT = V // COL_TILE                     # 8 tiles per block
CPT = COL_TILE // CHUNK                # chunk-max outputs per tile (250)
PREFETCH = 2                           # extra tile slots for the next block


class _Block:
    pass


def build_kernel(nc: bass.Bass):
    x = nc.dram_tensor("x", [ROWS_PER_CORE, V], f32, kind="ExternalInput").ap()
    y = nc.dram_tensor("y", [ROWS_PER_CORE, V], f32, kind="ExternalOutput").ap()

    with TileContext(nc) as tc:
        with (
            tc.tile_pool(name="data", bufs=NT) as data_pool,
            tc.tile_pool(name="mbuf", bufs=1) as mpool,
            tc.tile_pool(name="small", bufs=2) as spool,
            tc.tile_pool(name="psum", bufs=1, space="PSUM") as ppool,
            tc.tile_pool(name="ybuf", bufs=3) as ypool,
        ):
            def sm(tag, cols=1):
                return spool.tile([P, cols], f32, tag=tag, name=tag)

            def new_block(b):
                s = _Block()
                s.rows = slice(b * P, (b + 1) * P)
                s.xt = []
                s.M = mpool.tile([P, NCHUNKS], f32, tag="M", name="M")
                return s

            def load_tile(s, t, engine, tag="xcol"):
                cs = slice(t * COL_TILE, (t + 1) * COL_TILE)
                xt = data_pool.tile([P, COL_TILE], f32, tag=tag, name=tag,
                                    bufs=PREFETCH if tag == "xcol_pf" else None)
                s.xt.append(xt)
                engine.dma_start(out=xt, in_=x[s.rows, cs])
                view = xt.rearrange("p (c k) -> p c k", k=CHUNK)
                nc.vector.tensor_reduce(
                    out=s.M[:, t * CPT:(t + 1) * CPT],
                    in_=view, axis=AxX, op=Alu.max,
                )

            def topk_warm(s):
                VK = spool.tile([P, TOPK], f32, tag="VK", name="VK")
                for r in range(TOPK_ROUNDS):
                    nc.vector.max(out=VK[:, r * 8:(r + 1) * 8], in_=s.M)
                    if r + 1 < TOPK_ROUNDS:
                        nc.vector.match_replace(
                            out=s.M, in_to_replace=VK[:, r * 8:(r + 1) * 8],
                            in_values=s.M, imm_value=-1e30,
                        )
                mrow = sm("mrow")
                nc.vector.tensor_copy(mrow, VK[:, 0:1])  # top-1 == row max
                # Xs units: VK <- 0.5*VK - 0.5*mrow
                mh = sm("mh")
                nc.vector.tensor_scalar_mul(mh, mrow, 0.5)
                nc.vector.tensor_scalar(out=VK, in0=VK, scalar1=0.5, scalar2=mh,
                                        op0=Alu.mult, op1=Alu.subtract)
                z0 = sm("z0")
                nc.vector.memset(z0, 0.0)
                s.z0 = z0
                tau = sm("tau")
                nc.vector.memset(tau, -1.0)
                rV = spool.tile([P, TOPK], f32, tag="rV", name="rV")
                rV2 = spool.tile([P, TOPK], f32, tag="rV2", name="rV2")
                for _ in range(WARM_ITERS):
                    ws1, ws2, wrs, wst = sm("ws1"), sm("ws2"), sm("wrs"), sm("wst")
                    nc.vector.scalar_tensor_tensor(
                        out=rV, in0=VK, scalar=tau, in1=z0.to_broadcast([P, TOPK]),
                        op0=Alu.subtract, op1=Alu.max, accum_out=ws1,
                    )
                    nc.vector.tensor_mul(rV2, rV, rV)
                    nc.vector.tensor_reduce(out=ws2, in_=rV2, axis=AxX, op=Alu.add)
                    nc.vector.reciprocal(wrs, ws1)
                    nc.vector.scalar_tensor_tensor(
                        out=wst, in0=ws2, scalar=1.0, in1=wrs,
                        op0=Alu.subtract, op1=Alu.mult,
                    )
                    nc.vector.scalar_tensor_tensor(
                        out=tau, in0=wst, scalar=0.5, in1=tau,
                        op0=Alu.mult, op1=Alu.add,
                    )
                nc.vector.tensor_scalar(out=tau, in0=tau, scalar1=-1e-6,
                                        scalar2=None, op0=Alu.min)
                ctau = sm("ctau")
                nc.vector.tensor_scalar(out=ctau, in0=tau, scalar1=2.0,
                                        scalar2=mrow, op0=Alu.mult, op1=Alu.add)
                s.ctau = ctau

            def square_tile(s, t, sig2c):
                psq = ppool.tile([P, COL_TILE], f32, tag="psq", name="psq")
                nc.scalar.activation(
                    out=psq, in_=s.xt[t], func=Act.Square, scale=0.5,
                    accum_out=sig2c[:, t:t + 1],
                )

            def iter1(s):
                """Newton: r0 = relu(x - ctau) in place; s1, f0 accumulated."""
                sig1c = spool.tile([P, NT], f32, tag="sig1c", name="sig1c")
                sig2c = spool.tile([P, NT], f32, tag="sig2c", name="sig2c")
                for t in range(NT):
                    nc.vector.scalar_tensor_tensor(
                        out=s.xt[t], in0=s.xt[t], scalar=s.ctau,
                        in1=s.z0.to_broadcast([P, COL_TILE]),
                        op0=Alu.subtract, op1=Alu.max,
                        accum_out=sig1c[:, t:t + 1],
                    )
                    square_tile(s, t, sig2c)
                sig1, f0, rs, t0 = sm("sig1"), sm("f0"), sm("rs"), sm("t0")
                nc.vector.tensor_reduce(out=sig1, in_=sig1c, axis=AxX, op=Alu.add)
                nc.vector.tensor_reduce(out=f0, in_=sig2c, axis=AxX, op=Alu.add)
                nc.vector.reciprocal(rs, sig1)
                nc.vector.scalar_tensor_tensor(out=t0, in0=f0, scalar=1.0, in1=rs,
                                               op0=Alu.subtract, op1=Alu.mult)
                dX1 = sm("dX1")
                nc.vector.tensor_scalar(out=dX1, in0=t0, scalar1=2.0, scalar2=0.0,
                                        op0=Alu.mult, op1=Alu.max)
                s.f0, s.dX1 = f0, dX1

            def iter2(s):
                """Guarded secant: r1 = relu(r0 - dX1), f1, dX2 = clip(ratio)*dX1."""
                sig2c2 = spool.tile([P, NT], f32, tag="sig2c2", name="sig2c2")
                for t in range(NT):
                    nc.vector.tensor_scalar(
                        out=s.xt[t], in0=s.xt[t], scalar1=s.dX1, scalar2=0.0,
                        op0=Alu.subtract, op1=Alu.max,
                    )
                    square_tile(s, t, sig2c2)
                f1, den, rd, rat, dX2 = sm("f1"), sm("den"), sm("rd"), sm("rat"), sm("dX2")
                nc.vector.tensor_reduce(out=f1, in_=sig2c2, axis=AxX, op=Alu.add)
                nc.vector.tensor_sub(den, s.f0, f1)
                nc.vector.tensor_scalar(out=den, in0=den, scalar1=1e-20,
                                        scalar2=None, op0=Alu.max)
                nc.vector.reciprocal(rd, den)
                nc.vector.scalar_tensor_tensor(out=rat, in0=f1, scalar=1.0, in1=rd,
                                               op0=Alu.subtract, op1=Alu.mult)
                nc.vector.tensor_scalar(out=rat, in0=rat, scalar1=8.0, scalar2=0.0,
                                        op0=Alu.min, op1=Alu.max)
                nc.vector.tensor_mul(dX2, rat, s.dX1)
                s.dX2 = dX2

            def output_tile(s, t, nh, dX2=None):
                cs = slice(t * COL_TILE, (t + 1) * COL_TILE)
                if dX2 is not None and t >= NT - 2:
                    # DVE path: (r - dX2)*0.5 then square; runs concurrently
                    # with the ScalarE squares of the earlier tiles
                    nc.vector.tensor_scalar(out=s.xt[t], in0=s.xt[t],
                                            scalar1=dX2, scalar2=0.5,
                                            op0=Alu.subtract, op1=Alu.mult)
                    nc.vector.tensor_mul(s.xt[t], s.xt[t], s.xt[t])
                    nc.sync.dma_start(out=y[s.rows, cs], in_=s.xt[t])
                else:
                    # bounce through ybuf halves: the input tile's slot frees
                    # at Square time instead of output-DMA-completion time
                    H = COL_TILE // 2
                    for h in range(2):
                        yb = ypool.tile([P, H], f32, tag="yb", name="yb")
                        nc.scalar.activation(
                            out=yb, in_=s.xt[t][:, h * H:(h + 1) * H],
                            func=Act.Square, scale=0.5, bias=nh)
                        lo = t * COL_TILE + h * H
                        nc.sync.dma_start(out=y[s.rows, lo:lo + H], in_=yb)

            # ---------------- schedule ----------------
            s0 = new_block(0)
            for t in range(NT):
                # split the cold-start load across both DMA queue types
                load_tile(s0, t, nc.gpsimd if t % 2 else nc.sync)
            topk_warm(s0)
            iter1(s0)
            iter2(s0)

            s1 = new_block(1)
            for t in range(PREFETCH):   # dedicated slots: stream in early
                load_tile(s1, t, nc.gpsimd, tag="xcol_pf")

            nh0 = sm("nh")
            nc.vector.tensor_scalar_mul(nh0, s0.dX2, -0.5)
            for t in range(NT):
                output_tile(s0, t, nh0, s0.dX2)
                if t + PREFETCH < NT:
                    load_tile(s1, t + PREFETCH, nc.gpsimd)
            topk_warm(s1)
            iter1(s1)
            iter2(s1)
            nh1 = sm("nh1")
            nc.vector.tensor_scalar_mul(nh1, s1.dX2, -0.5)
            for t in range(NT):
                output_tile(s1, t, nh1, s1.dX2)
    return nc


_COMPILED = {}


def _get_nc():
    if "nc" not in _COMPILED:
        nc = bacc.Bacc("TRN2", target_bir_lowering=False, debug=False,
                       num_devices=N_CORES)
        build_kernel(nc)
        nc.compile()
        _COMPILED["nc"] = nc
    return _COMPILED["nc"]


def kernel(X: np.ndarray) -> np.ndarray:
    assert X.shape == (ROWS_TOTAL, V) and X.dtype == np.float32, (X.shape, X.dtype)
    nc = _get_nc()
    in_maps = [
        {"x": np.ascontiguousarray(X[i * ROWS_PER_CORE:(i + 1) * ROWS_PER_CORE])}
        for i in range(N_CORES)
    ]
    res = run_bass_kernel_spmd(nc, in_maps, core_ids=list(range(N_CORES)))
    return np.concatenate([r["y"] for r in res.results], axis=0)



# revision 4
# speedup vs baseline: 1.9513x; 1.9513x over previous
"""Entmax-1.5 over rows of a (2048, 32000) fp32 tensor on 8 Trainium2 NeuronCores.

Algorithm per row (raw-units threshold c, y = relu((x - c)/2)^2, sum y = 1):
  1. cast-DMA loads x as fp16 tiles (SWDGE fp32->fp16), DVE TT-max fold
     chain reduces each 128-row block to M[1000] strided chunk maxima.
  2. 8x subrange top-8 (DVE max8) -> 64 warm candidates per row.
  3. 5 warm Newton iters on the candidates (fp32 small ops) -> c_w and
     sig_warm = sum relu(v - c_w) over candidates (~ the full derivative,
     validated numerically: underestimating it slightly overshoots the
     from-below Newton step toward the true root).
  4. relu pass in place (DVE tensor_scalar 4x fp16, no accum).
  5. f0 = sum (r/2)^2 via ScalarE Square accum (output -> PSUM trash).
  6. dc = max(0, (f0-1)*2/sig_warm); out pass split between ScalarE
     (Square with folded bias -> fp16 bounce buf) and DVE (shift+self-mult
     in place), both DMA to fp16 DRAM output.

Host: shard rows 8 ways, gather, cast fp16 -> fp32.
Validated on host sim vs float64 reference: rel err ~1.7e-3 (gate 2e-2).
"""

import numpy as np

import concourse.bass as bass
import concourse.bacc as bacc
import concourse.mybir as mybir
from concourse.tile import TileContext
from concourse.bass_utils import run_bass_kernel_spmd

f32 = mybir.dt.float32
f16 = mybir.dt.float16
Alu = mybir.AluOpType
Act = mybir.ActivationFunctionType
AxX = mybir.AxisListType.X

ROWS_TOTAL = 2048
V = 32000
N_CORES = 8
ROWS_PER_CORE = ROWS_TOTAL // N_CORES  # 256
P = 128
N_BLOCKS = ROWS_PER_CORE // P          # 2
TILE_W = 8000
NT = V // TILE_W                       # 4 tiles per block
MW = 1000                              # chunk-maxima width
NRANGE = 8                             # subranges for top-8 candidates
K = 8 * NRANGE                         # 64 warm candidates
WARM_ITERS = 5
SCALAR_OUT_TILES = 2                   # tiles 0..1 on ScalarE, rest on DVE


class _Blk:
    pass


def build_kernel(nc: bass.Bass):
    x = nc.dram_tensor("x", [ROWS_PER_CORE, V], f32, kind="ExternalInput").ap()
    y = nc.dram_tensor("y", [ROWS_PER_CORE, V], f16, kind="ExternalOutput").ap()

    with TileContext(nc) as tc:
        with (
            tc.tile_pool(name="data", bufs=2 * NT) as dpool,
            tc.tile_pool(name="fold", bufs=2) as gpool,
            tc.tile_pool(name="ybuf", bufs=2) as ypool,
            tc.tile_pool(name="small", bufs=2) as spool,
            tc.tile_pool(name="psum", bufs=1, space="PSUM") as ppool,
        ):
            def sm(tag, cols=1, dt=f32):
                return spool.tile([P, cols], dt, tag=tag, name=tag)

            def new_block(b):
                s = _Blk()
                s.rows = slice(b * P, (b + 1) * P)
                s.xt = []
                return s

            def load_fold(s, name):
                with nc.named_scope(f"load{name}"):
                    for t in range(NT):
                        xt = dpool.tile([P, TILE_W], f16, tag="xt", name="xt")
                        s.xt.append(xt)
                        cs = slice(t * TILE_W, (t + 1) * TILE_W)
                        nc.gpsimd.dma_start(out=xt, in_=x[s.rows, cs])
                    G = gpool.tile([P, TILE_W], f16, tag="G", name="G")
                    s.G = G
                    nc.vector.tensor_tensor(out=G, in0=s.xt[0], in1=s.xt[1],
                                            op=Alu.max)
                    for t in range(2, NT):
                        nc.vector.tensor_tensor(out=G, in0=G, in1=s.xt[t],
                                                op=Alu.max)
                    w = TILE_W
                    while w > MW:
                        h = w // 2
                        nc.vector.tensor_tensor(out=G[:, 0:h], in0=G[:, 0:h],
                                                in1=G[:, h:w], op=Alu.max)
                        w = h

            def warm(s, name):
                with nc.named_scope(f"warm{name}"):
                    VK = sm(f"VK", K, f16)
                    W = MW // NRANGE
                    for i in range(NRANGE):
                        nc.vector.max(out=VK[:, 8 * i:8 * i + 8],
                                      in_=s.G[:, W * i:W * (i + 1)])
                    VKf = sm("VKf", K)
                    nc.vector.tensor_copy(VKf, VK)
                    vmax = sm("vmax")
                    nc.vector.tensor_reduce(out=vmax, in_=VKf, axis=AxX,
                                            op=Alu.max)
                    c = sm("c")
                    nc.vector.tensor_scalar(out=c, in0=vmax, scalar1=2.0,
                                            scalar2=None, op0=Alu.subtract)
                    z0 = sm("z0")
                    nc.vector.memset(z0, 0.0)
                    zb = z0.to_broadcast([P, K])
                    rV = sm("rV", K)
                    rV2 = sm("rV2", K)
                    sig, q, rs, u = sm("sig"), sm("q"), sm("rs"), sm("u")
                    for _ in range(WARM_ITERS):
                        nc.vector.scalar_tensor_tensor(
                            out=rV, in0=VKf, scalar=c, in1=zb,
                            op0=Alu.subtract, op1=Alu.max, accum_out=sig)
                        nc.vector.scalar_tensor_tensor(
                            out=rV2, in0=rV, scalar=1.0, in1=rV,
                            op0=Alu.mult, op1=Alu.mult, accum_out=q)
                        nc.vector.reciprocal(rs, sig)
                        nc.vector.scalar_tensor_tensor(
                            out=u, in0=q, scalar=4.0, in1=rs,
                            op0=Alu.subtract, op1=Alu.mult)
                        nc.vector.tensor_scalar(out=c, in0=u, scalar1=0.5,
                                                scalar2=c, op0=Alu.mult,
                                                op1=Alu.add)
                    cw = sm("cw")
                    # cw = min(vmax - 1e-6, c)
                    nc.vector.tensor_scalar(out=cw, in0=vmax, scalar1=1e-6,
                                            scalar2=c, op0=Alu.subtract,
                                            op1=Alu.min)
                    sigw = sm("sigw")
                    nc.vector.scalar_tensor_tensor(
                        out=rV, in0=VKf, scalar=cw, in1=zb,
                        op0=Alu.subtract, op1=Alu.max, accum_out=sigw)
                    rsig = sm("rsig")
                    nc.vector.reciprocal(rsig, sigw)
                    s.cw, s.rsig = cw, rsig

            def relu_f0(s, name):
                with nc.named_scope(f"iter{name}"):
                    f0c = sm("f0c", 2 * NT)
                    for t in range(NT):
                        nc.vector.tensor_scalar(out=s.xt[t], in0=s.xt[t],
                                                scalar1=s.cw, scalar2=0.0,
                                                op0=Alu.subtract, op1=Alu.max)
                        for h in range(2):
                            ps = ppool.tile([P, TILE_W // 2], f32, tag="ps",
                                            name="ps")
                            nc.scalar.activation(
                                out=ps,
                                in_=s.xt[t][:, h * 4000:(h + 1) * 4000],
                                func=Act.Square, scale=0.5,
                                accum_out=f0c[:, 2 * t + h:2 * t + h + 1])
                    f0 = sm("f0")
                    nc.vector.tensor_reduce(out=f0, in_=f0c, axis=AxX,
                                            op=Alu.add)
                    # dc = max(0, (f0 - 1) * 2 * rsig); nh = -dc/2
                    dc0, dc, nh = sm("dc0"), sm("dc"), sm("nh")
                    nc.vector.scalar_tensor_tensor(
                        out=dc0, in0=f0, scalar=1.0, in1=s.rsig,
                        op0=Alu.subtract, op1=Alu.mult)
                    nc.vector.tensor_scalar(out=dc, in0=dc0, scalar1=2.0,
                                            scalar2=0.0, op0=Alu.mult,
                                            op1=Alu.max)
                    nc.vector.tensor_scalar(out=nh, in0=dc, scalar1=-0.5,
                                            scalar2=None, op0=Alu.mult)
                    s.dc, s.nh = dc, nh

            def out_pass(s, name):
                with nc.named_scope(f"out{name}"):
                    for t in range(NT):
                        cs = slice(t * TILE_W, (t + 1) * TILE_W)
                        if t < SCALAR_OUT_TILES:
                            yb = ypool.tile([P, TILE_W], f16, tag="yb",
                                            name="yb")
                            nc.scalar.activation(out=yb, in_=s.xt[t],
                                                 func=Act.Square, scale=0.5,
                                                 bias=s.nh)
                            nc.sync.dma_start(out=y[s.rows, cs], in_=yb)
                        else:
                            nc.vector.tensor_scalar(
                                out=s.xt[t], in0=s.xt[t], scalar1=s.dc,
                                scalar2=0.5, op0=Alu.subtract, op1=Alu.mult)
                            nc.vector.tensor_tensor(out=s.xt[t], in0=s.xt[t],
                                                    in1=s.xt[t], op=Alu.mult)
                            nc.sync.dma_start(out=y[s.rows, cs], in_=s.xt[t])

            blocks = [new_block(b) for b in range(N_BLOCKS)]
            names = ["A", "B"]
            for b, s in enumerate(blocks):
                load_fold(s, names[b])
            for b, s in enumerate(blocks):
                warm(s, names[b])
                relu_f0(s, names[b])
                out_pass(s, names[b])
    return nc


_COMPILED = {}


def _get_nc():
    if "nc" not in _COMPILED:
        nc = bacc.Bacc("TRN2", target_bir_lowering=False, debug=False,
                       num_devices=N_CORES)
        build_kernel(nc)
        nc.compile()
        _COMPILED["nc"] = nc
    return _COMPILED["nc"]


def kernel(X: np.ndarray) -> np.ndarray:
    assert X.shape == (ROWS_TOTAL, V) and X.dtype == np.float32, (X.shape, X.dtype)
    nc = _get_nc()
    in_maps = [
        {"x": np.ascontiguousarray(X[i * ROWS_PER_CORE:(i + 1) * ROWS_PER_CORE])}
        for i in range(N_CORES)
    ]
    res = run_bass_kernel_spmd(nc, in_maps, core_ids=list(range(N_CORES)))
    return np.concatenate(
        [r["y"].astype(np.float32) for r in res.results], axis=0)
